# revision 27
# baseline (speedup 1.0000x reference)
"""AtomPosGNN distributed Trainium2 kernel (8 NeuronCores).

Reference computation (N=8192 nodes, H=128 features, L=4 layers):
    feat = concat(atom_pos, atom_emb)            # [N, 128]
    deg = dist_adj.sum(-1); isd = rsqrt(deg)
    for l in range(4):
        h = (feat * isd[:, None]) @ Ws[l]
        h = dist_adj @ h
        feat = softplus(h * isd[:, None] + bs[l])

Strategy (row shard, P=1024 rows per core, transpose-free):
  - Host passes each core its adj row-block transposed, tiled, and cast
    to bf16 in partition-major layout [128, KB, P] so every DMA
    partition line is a long contiguous run.
  - Prep: adjT streamed on the sync+scalar HWDGE queues, big chunks
    first (the DMA engines round-robin across in-flight transfers, so
    small tail chunks keep the deg backlog short); deg = ones-stationary
    matmul pass riding behind the DMA; isd = Exp(-0.5*Ln(deg)) on ACT
    (both funcs in ONE table set, see the get_activation_tables patch),
    then PE ones-broadcast to [128,P]. g0 for both halves is gathered in
    a single AllGather (both pieces are ready at the same time).
  - Per layer the 1024 output columns run in two 512-col passes. After
    pass A its epilogue (DVE mult + ACT Exp/Ln softplus) + local g +
    AllGather (128KB) run under pass B at high scheduler priority; the
    next layer consumes k-tiles in matching half order so only the
    B-half AllGather tail is exposed (~10us: mesh latency ~14us vs the
    9.5us consumption offset; structural for collective_compute).
  - Queue discipline: AG staging + g-copy ride the scalar/ACT queue
    (idle between epilogues), gathered-g loads ride sync split r0-3 /
    r4-7 (consumers iterate r-inner), collectives trigger from gpsimd.
    Never queue a sem-waiting DMA ahead of latency-critical work.
  - adj is read from HBM exactly once; layers run entirely from SBUF.
  - Last layer's epilogue runs in 256-col sub-chunks so the final
    writeout DMA tail is short.
"""

import os
import sys

for _p in ("/opt/trn_rl_repo",):
    if _p not in sys.path and os.path.isdir(_p):
        sys.path.insert(0, _p)

import numpy as np
import ml_dtypes

import concourse.bacc as bacc
import concourse.bass as bass
import concourse.mybir as mybir
import concourse.tile as tile
from concourse.bass_utils import run_bass_kernel_spmd

# Both Exp and Ln live in the natural_log_exp_and_others ACT table set,
# but the table chooser resolves each func to the first set containing
# it, inserting a 1.28us ACT_TABLE_LOAD between every Exp and Ln. Strip
# exp/ln from every other set (positions preserved, so act_func_set_id
# stays aligned with act_info.json) so the whole kernel runs off one
# table with zero switches.
if os.environ.get("K_TBL", "1") == "1":
    import concourse.hw_specs as _hw_specs

    _orig_gat = _hw_specs.get_activation_tables

    def _patched_gat(arch):
        tabs = _orig_gat(arch)
        keep = "natural_log_exp_and_others"
        if keep in tabs:
            drop = {
                mybir.ActivationFunctionType.Exp,
                mybir.ActivationFunctionType.Ln,
            }
            for name, funcs in tabs.items():
                if name != keep:
                    funcs -= drop
        return tabs

    _hw_specs.get_activation_tables = _patched_gat
    bacc.get_activation_tables = _patched_gat

R = 8          # cores
N = 8192       # nodes
P = N // R     # local rows = 1024
H = 128        # hidden
L = 4          # layers
KB = N // 128  # 64 k-tiles

F32 = mybir.dt.float32
BF16 = mybir.dt.bfloat16

WARM_AG = os.environ.get("K_WARM", "1") == "1"
ISD_MODE = os.environ.get("K_ISD", "lnexp")   # arsqrt | lnexp | dve
SP_MODE = os.environ.get("K_SP", "expln")     # expln | manual
# XOR-relative remote_dma exchange instead of mesh AllGather for g.
# Cross-die rounds need dtpb = r^2 (physical nc map [0,1,2,3,6,7,4,5],
# measured by probe_rdma.py).
RDMA = os.environ.get("K_RDMA", "0") == "1"

LOG_A = float(np.log(2.0) / (1 << 23))
LOG_B = float(-np.log(2.0) * (127 + 0.0450466))

LAST_RESULT = None
_NC_CACHE = {}

# column/row halves: (lo, hi) in local node index
GROUPS = [(0, 512), (512, 1024)]
# adj chunk widths (k-tiles per dma_start). The DMA engines round-robin
# across in-flight transfers, so every chunk completes near the END of
# the load window; big-first ordering keeps the final chunks small so
# the deg matmul backlog after the last arrival is tiny.
CHUNKS = [8, 8, 8, 8, 8, 8, 8, 4, 2, 1, 1]
assert sum(CHUNKS) == KB


def _softplus_manual(nc, sp_pool, out_ap, in_ap, bias_ap, hw):
    """out = softplus(in + bias), composed (fallback path)."""
    z0 = sp_pool.tile([H, hw], F32, name="z0", tag=f"sp_b{hw}")
    nc.scalar.activation(
        z0[:, :], in_ap, mybir.ActivationFunctionType.Exp, bias=bias_ap, scale=1.0
    )
    z = sp_pool.tile([H, hw], F32, name="z", tag=f"sp_c{hw}")
    nc.vector.tensor_scalar_add(z[:, :], z0[:, :], 1.0)
    y0 = sp_pool.tile([H, hw], F32, name="y0", tag=f"sp_d{hw}")
    nc.vector.tensor_scalar(
        y0[:, :], z[:, :].bitcast(mybir.dt.int32), LOG_A, LOG_B,
        mybir.AluOpType.mult, mybir.AluOpType.add,
    )
    w_e = sp_pool.tile([H, hw], F32, name="w_e", tag=f"sp_e{hw}")
    nc.scalar.activation(
        w_e[:, :], y0[:, :], mybir.ActivationFunctionType.Exp, scale=-1.0
    )
    t1 = sp_pool.tile([H, hw], F32, name="t1", tag=f"sp_f{hw}")
    nc.vector.tensor_tensor(t1[:, :], z[:, :], w_e[:, :], mybir.AluOpType.mult)
    nc.vector.tensor_scalar_add(t1[:, :], t1[:, :], -1.0)
    nc.vector.tensor_tensor(out_ap, t1[:, :], y0[:, :], mybir.AluOpType.add)


def build_nc():
    nc = bacc.Bacc("TRN2", target_bir_lowering=False, debug=False, num_devices=R)

    adjT_ext = nc.declare_dram_parameter("adjT", [128, KB, P], BF16, isOutput=False)
    featT_ext = nc.declare_dram_parameter("featT", [H, P], F32, isOutput=False)
    ws_ext = nc.declare_dram_parameter("ws", [L, H, H], BF16, isOutput=False)
    bsT_ext = nc.declare_dram_parameter("bsT", [H, L], F32, isOutput=False)
    out_ext = nc.declare_dram_parameter("out", [H, P], F32, isOutput=True)

    with tile.TileContext(nc) as tc:
        with (
            tc.tile_pool(name="big", bufs=1) as big,
            tc.tile_pool(name="sb", bufs=1) as sb,
            tc.tile_pool(name="ftl", bufs=2) as ftl_pool,
            tc.tile_pool(name="pre", bufs=2) as pre_pool,
            tc.tile_pool(name="sp", bufs=1) as sp_pool,
            tc.tile_pool(name="gsb", bufs=2) as gsb_pool,
            tc.tile_pool(name="psd", bufs=1, space="PSUM") as psd,
            tc.tile_pool(name="psg", bufs=2, space="PSUM") as psg,
            tc.tile_pool(name="psy", bufs=1, space="PSUM") as psy,
            tc.tile_pool(name="dram", bufs=1, space="DRAM") as dram,
        ):
            # RDMA exchange state: one remote sem per half-exchange
            # (8 exchanges: g_l halves, ex = 2l+gi), one local sem for
            # send-buffer reuse. Waits are emitted with threshold 0 (the
            # single-core scheduling sim cannot model remote arrivals)
            # and patched to the real values after scheduling.
            wpatch = []
            if RDMA:
                rsems = [nc.alloc_semaphore(f"rs{e}") for e in range(8)]
                lsem = nc.alloc_semaphore("lsem")

            def rdma_exchange(ex, g_stage_ap, dst_ap):
                """8 XOR-round sends of g_stage to every core's dst slot.

                Round r reaches logical peer me^r; cross-die rounds need
                the physical-map correction dtpb = r^2. Receiver slot for
                sender s is s^me = r, so out_ap slot r is compile-time.
                """
                for r in range(R):
                    dt = (r ^ 2) if r >= 4 else r
                    rd = [None] * 8
                    rd[dt] = (0, dt)
                    nc.gpsimd.remote_dma_broadcast(
                        out_ap=dst_ap(r),
                        in_ap=g_stage_ap,
                        remote_sem=rsems[ex],
                        local_sem=lsem,
                        rdests=rd,
                    )
                nc.gpsimd.trigger_dma(count=None)

            # warm the collective path at the very start: the first
            # collective pays a large cold staging cost.
            if WARM_AG and not RDMA:
                warm_in = dram.tile([8, H], BF16, name="warm_in")
                warm_out = dram.tile([8 * R, H], BF16, addr_space="Shared", name="warm_out")
                nc.gpsimd.collective_compute(
                    "AllGather",
                    mybir.AluOpType.bypass,
                    replica_groups=[list(range(R))],
                    ins=[warm_in[:, :]],
                    outs=[warm_out[:, :]],
                )

            # ---- persistent SBUF ----
            at = big.tile([128, KB, P], BF16, name="at")  # adjT resident
            ones = sb.tile([128, 1], BF16, name="ones")
            nc.vector.memset(ones[:, :], 1.0)
            w_sb = sb.tile([128, L, H], BF16, name="w_sb")
            nc.scalar.dma_start(
                out=w_sb[:, :, :], in_=ws_ext.rearrange("l k h -> k l h")
            )
            bsT_sb = sb.tile([H, L], F32, name="bsT_sb")
            nc.scalar.dma_start(out=bsT_sb[:, :], in_=bsT_ext[:, :])
            ftl0 = ftl_pool.tile([H, P], F32, name="ftl", tag="ftl")
            nc.scalar.dma_start(out=ftl0[:, :], in_=featT_ext[:, :])

            # ---- prep: load adjT on the two HWDGE queues ----
            # partition-major host layout: each partition line is one
            # contiguous run of w*2KB
            qeng = [nc.sync, nc.scalar]
            kb0 = 0
            for ci, w in enumerate(CHUNKS):
                qeng[ci % 2].dma_start(
                    out=at[:, kb0 : kb0 + w, :],
                    in_=adjT_ext[:, kb0 : kb0 + w, :],
                )
                kb0 += w

            # deg[r] = sum_j adjT[j, r]: ones-stationary matmul pass
            deg_ps = psd.tile([1, 2, 512], F32, name="deg_ps")
            for kb in range(KB):
                for hh in range(2):
                    nc.tensor.matmul(
                        deg_ps[:, hh, :],
                        ones[:, :],
                        at[:, kb, hh * 512 : (hh + 1) * 512],
                        start=(kb == 0),
                        stop=(kb == KB - 1),
                    )

            # isd = rsqrt(deg) on one partition (ACT table), then
            # broadcast to 128 partitions with a K=1 ones matmul.
            # High priority: it gates the first AllGather.
            ones_row = sb.tile([1, 128], F32, name="ones_row")
            isd_rep = sb.tile([128, P], F32, name="isd_rep")
            deg_flat = deg_ps.rearrange("o h x -> o (h x)")
            with tc.high_priority():
                isd_row = sb.tile([1, P], F32, name="isd_row")
                if ISD_MODE == "arsqrt":
                    # |x|^-1/2 table == rsqrt for positive deg
                    nc.vector.memset(ones_row[:, :], 1.0)
                    nc.scalar.activation(
                        isd_row[0:1, :], deg_flat,
                        mybir.ActivationFunctionType.Abs_reciprocal_sqrt,
                    )
                elif ISD_MODE == "lnexp":
                    # isd = exp(-0.5 * ln(deg))
                    nc.vector.memset(ones_row[:, :], 1.0)
                    t0 = sb.tile([1, P], F32, name="isd_t0")
                    nc.scalar.activation(
                        t0[0:1, :], deg_flat, mybir.ActivationFunctionType.Ln
                    )
                    nc.scalar.activation(
                        isd_row[0:1, :], t0[0:1, :],
                        mybir.ActivationFunctionType.Exp, scale=-0.5,
                    )
                else:
                    nc.vector.memset(ones_row[:, :], 1.0)
                    nc.vector.reciprocal(isd_row[0:1, :], deg_flat)
                    nc.scalar.sqrt(isd_row[0:1, :], isd_row[0:1, :])
                dbc_ps = psd.tile([128, 2, 512], F32, name="dbc_ps")
                for hh in range(2):
                    nc.tensor.matmul(
                        dbc_ps[:, hh, :],
                        ones_row[:, :],
                        isd_row[0:1, hh * 512 : (hh + 1) * 512],
                        start=True,
                        stop=True,
                    )
                nc.vector.tensor_copy(
                    isd_rep[:, :], dbc_ps.rearrange("p h x -> p (h x)")
                )

            def make_g(l, gi, ftl_src, isd_src=None, g_dst=None):
                """Local g rows [lo, hi) -> DRAM, AllGather; returns AG out.

                Staging DMA rides the scalar queue so it is never stuck
                behind a sem-waiting gathered-g load on sync. g_in is
                partition-major so the AG payload moves in 1KB runs.
                """
                lo, hi = GROUPS[gi]
                kw = (hi - lo) // 128
                if isd_src is None:
                    isd_src = isd_rep
                with tc.high_priority():
                    ftl_s = pre_pool.tile(
                        [H, hi - lo], BF16, name="ftl_s", tag=f"ftls{gi}"
                    )
                    nc.vector.tensor_tensor(
                        ftl_s[:, :], ftl_src[:, lo:hi], isd_src[:, lo:hi],
                        mybir.AluOpType.mult,
                    )
                    g_ps = psg.tile([128, kw, H], F32, name="g_ps", tag="gps")
                    for nb in range(kw):
                        nc.tensor.matmul(
                            g_ps[:, nb, :],
                            ftl_s[:, nb * 128 : (nb + 1) * 128],
                            w_sb[:, l, :],
                            start=True,
                            stop=True,
                        )
                    ex = 2 * l + gi
                    if RDMA and ex >= 4:
                        # this g_stage buffer (tag gi, bufs=2) was last
                        # used by exchange ex-4; its lane reads must have
                        # drained (each exchange bumps lsem by 8*16=128)
                        wpatch.append(
                            (nc.scalar.wait_ge(lsem, 0), 128 * (ex - 3))
                        )
                    g_stage = pre_pool.tile(
                        [128, kw, H], BF16, name="g_stage", tag=f"gstage{gi}"
                    )
                    # ACT does the PSUM->bf16 copy: DVE is busy with the
                    # epilogue mults at this point, ACT is idle (Copy is
                    # in every table set, so no table switch)
                    nc.scalar.activation(
                        g_stage[:, :, :], g_ps[:, :, :],
                        mybir.ActivationFunctionType.Copy,
                    )
                    if RDMA:
                        rdma_exchange(
                            ex, g_stage[:, :, :],
                            lambda r: g_dst[:, gi, r, :, :],
                        )
                        return None
                    g_in = dram.tile([128, kw * H], BF16, name=f"g_in{l}_{gi}")
                    nc.scalar.dma_start(
                        out=g_in.rearrange("p (k h) -> p k h", k=kw),
                        in_=g_stage[:, :, :],
                    )
                    g_out = dram.tile(
                        [R * 128, kw * H], BF16, addr_space="Shared",
                        name=f"g_out{l}_{gi}",
                    )
                    nc.gpsimd.collective_compute(
                        "AllGather",
                        mybir.AluOpType.bypass,
                        replica_groups=[list(range(R))],
                        ins=[g_in[:, :]],
                        outs=[g_out[:, :]],
                    )
                return g_out

            def load_g(g_sb, gi, g_out, split=False):
                """Gathered g -> SBUF [128, 2, R, 4, H]; 1KB runs.

                Two parts so ranks 0-3 land first (the consuming blocks
                iterate r-inner); split=True additionally puts the
                second part on scalar (prep only -- in steady state that
                would head-of-line-block the epilogue ACT ops).
                """
                lo, hi = GROUPS[gi]
                kw = (hi - lo) // 128
                g_out_r = g_out.rearrange(
                    "(r p) (k h) -> p r k h", p=128, k=kw
                )
                eng2 = nc.scalar if split else nc.sync
                nc.sync.dma_start(
                    out=g_sb[:, gi, 0:4, :, :],
                    in_=g_out_r[:, 0:4, :, :],
                )
                eng2.dma_start(
                    out=g_sb[:, gi, 4:8, :, :],
                    in_=g_out_r[:, 4:8, :, :],
                )

            # ---- layer 0 g ----
            # Both halves of g0 are ready at the same time (prep), so
            # gather them in ONE mesh instead of two serialized ones.
            # isd is read straight from the broadcast PSUM (dbc) so g0
            # does not wait for the isd_rep SBUF copy.
            dbc_flat = dbc_ps.rearrange("p h x -> p (h x)")
            g_sb = gsb_pool.tile([128, 2, R, 4, H], BF16, name="g_sb", tag="gsb")
            with tc.high_priority():
                ftl_s0 = pre_pool.tile([H, P], BF16, name="ftl_s0", tag="ftls0")
                nc.vector.tensor_tensor(
                    ftl_s0[:, :], ftl0[:, :], dbc_flat, mybir.AluOpType.mult
                )
                g_stage0 = pre_pool.tile(
                    [128, 8, H], BF16, name="g_stage0", tag="gstage0"
                )
                for gi in range(2):
                    g_ps = psg.tile([128, 4, H], F32, name="g_ps", tag="gps")
                    for nb in range(4):
                        nc.tensor.matmul(
                            g_ps[:, nb, :],
                            ftl_s0[:, gi * 512 + nb * 128 : gi * 512 + (nb + 1) * 128],
                            w_sb[:, 0, :],
                            start=True,
                            stop=True,
                        )
                    nc.scalar.activation(
                        g_stage0[:, gi * 4 : gi * 4 + 4, :], g_ps[:, :, :],
                        mybir.ActivationFunctionType.Copy,
                    )
                if RDMA:
                    for gi in range(2):
                        rdma_exchange(
                            gi, g_stage0[:, gi * 4 : gi * 4 + 4, :],
                            lambda r, _gi=gi: g_sb[:, _gi, r, :, :],
                        )
                else:
                    for gi in range(2):
                        g_in0 = dram.tile([128, 4 * H], BF16, name=f"g_in0_{gi}")
                        nc.scalar.dma_start(
                            out=g_in0.rearrange("p (k h) -> p k h", k=4),
                            in_=g_stage0[:, gi * 4 : gi * 4 + 4, :],
                        )
                        g_out0 = dram.tile(
                            [R * 128, 4 * H], BF16, addr_space="Shared",
                            name=f"g_out0_{gi}",
                        )
                        nc.gpsimd.collective_compute(
                            "AllGather",
                            mybir.AluOpType.bypass,
                            replica_groups=[list(range(R))],
                            ins=[g_in0[:, :]],
                            outs=[g_out0[:, :]],
                        )
                        load_g(g_sb, gi, g_out0, split=True)

            ftl = ftl0
            # ---- layers ----
            for l in range(L):
                yt_ps = psy.tile([H, P], F32, name="yt_ps", tag="ytps")
                ftl_next = ftl_pool.tile([H, P], F32, name="ftl", tag="ftl")
                g_sb_next = (
                    gsb_pool.tile([128, 2, R, 4, H], BF16, name="g_sb", tag="gsb")
                    if l < L - 1
                    else None
                )

                for gi, (lo, hi) in enumerate(GROUPS):
                    i = 0
                    for tg in range(2):          # k-tile groups: A then B
                        if RDMA and gi == 0:
                            # gate this tile-group on its exchange: all 8
                            # sends (2 lane-credits each) have landed
                            wpatch.append(
                                (nc.tensor.wait_ge(rsems[2 * l + tg], 0), 16)
                            )
                        for k in range(tg * 4, tg * 4 + 4):
                            for r in range(R):
                                nc.tensor.matmul(
                                    yt_ps[:, lo:hi],
                                    g_sb[:, k // 4, r, k % 4, :],
                                    at[:, r * 8 + k, lo:hi],
                                    start=(i == 0),
                                    stop=(i == KB - 1),
                                )
                                # splice next layer's A-half g chain into
                                # the hh1 stream (its epilogue runs on
                                # DVE/ACT just after hh0 stopped)
                                if i == 12 and gi == 1 and l < L - 1:
                                    g_out = make_g(l + 1, 0, ftl_next, g_dst=g_sb_next)
                                    if not RDMA:
                                        load_g(g_sb_next, 0, g_out)
                                i += 1
                    # epilogue for this pass's columns (runs under next
                    # pass, high priority: it gates the AllGather chain).
                    # Last layer: 256-col sub-chunks so the final output
                    # DMA tail is short.
                    sub = 2 if l == L - 1 else 1
                    sw = 512 // sub
                    with tc.high_priority():
                        for si in range(sub):
                            slo = lo + si * sw
                            x1 = sp_pool.tile(
                                [H, sw], F32, name="x1", tag=f"sp_a{sw}"
                            )
                            nc.vector.tensor_tensor(
                                x1[:, :], yt_ps[:, slo : slo + sw],
                                isd_rep[:, slo : slo + sw],
                                mybir.AluOpType.mult,
                            )
                            if SP_MODE == "expln":
                                # softplus(x+b) = ln(1 + exp(x+b)); both
                                # funcs live in the natural_log_exp table
                                # set, so no ACT table switch anywhere
                                z0 = sp_pool.tile(
                                    [H, sw], F32, name="z0", tag=f"sp_b{sw}"
                                )
                                nc.scalar.activation(
                                    z0[:, :], x1[:, :],
                                    mybir.ActivationFunctionType.Exp,
                                    bias=bsT_sb[:, l : l + 1],
                                )
                                nc.scalar.activation(
                                    ftl_next[:, slo : slo + sw], z0[:, :],
                                    mybir.ActivationFunctionType.Ln,
                                    bias=1.0,
                                )
                            else:
                                _softplus_manual(
                                    nc, sp_pool, ftl_next[:, slo : slo + sw],
                                    x1[:, :], bsT_sb[:, l : l + 1], sw,
                                )
                            if l == L - 1:
                                qeng[(gi * sub + si) % 2].dma_start(
                                    out=out_ext[:, slo : slo + sw],
                                    in_=ftl_next[:, slo : slo + sw],
                                )
                # B-half g/AG (exposed tail, covered by next layer's A part)
                if l < L - 1:
                    g_out = make_g(l + 1, 1, ftl_next, g_dst=g_sb_next)
                    if not RDMA:
                        load_g(g_sb_next, 1, g_out)
                    g_sb = g_sb_next
                ftl = ftl_next

    patch_map = {w.ins.name: v for w, v in wpatch}
    if patch_map:
        n_patched = 0
        for b in nc.m.functions[0].blocks:
            for i in b.instructions:
                if i.name in patch_map:
                    for we in i.sync_info.on_wait:
                        if we.ant_name and (
                            we.ant_name.startswith("rs") or we.ant_name == "lsem"
                        ):
                            we.wait_value = patch_map[i.name]
                            n_patched += 1
        assert n_patched == len(patch_map), (n_patched, len(patch_map))
    nc.compile()
    return nc


def kernel(atom_pos, atom_emb, dist_adj, Ws, bs):
    global LAST_RESULT
    atom_pos = np.asarray(atom_pos, dtype=np.float32)
    atom_emb = np.asarray(atom_emb, dtype=np.float32)
    dist_adj = np.asarray(dist_adj, dtype=np.float32)
    Ws = np.asarray(Ws, dtype=np.float32)
    bs = np.asarray(bs, dtype=np.float32)

    feat = np.concatenate([atom_pos, atom_emb], axis=-1)  # [N, H]
    ws_bf = Ws.astype(ml_dtypes.bfloat16)
    bsT = np.ascontiguousarray(bs.T)  # [H, L]
    adj_bf = dist_adj.astype(ml_dtypes.bfloat16)

    if "nc" not in _NC_CACHE:
        _NC_CACHE["nc"] = build_nc()
    nc = _NC_CACHE["nc"]

    in_maps = []
    for c in range(R):
        rows = slice(c * P, (c + 1) * P)
        # [128, KB, P]: partition-major tiled transpose of the row block.
        # Under RDMA, slot s holds global rank s^c's tiles (the XOR
        # exchange lands sender s's data at receiver slot s^c).
        tiles = adj_bf[rows].T.reshape(KB, 128, P)
        if RDMA:
            idx = np.array(
                [(s ^ c) * 8 + k for s in range(R) for k in range(8)]
            )
            tiles = tiles[idx]
        adjT_c = np.ascontiguousarray(tiles.transpose(1, 0, 2))
        in_maps.append(
            {
                "adjT": adjT_c,
                "featT": np.ascontiguousarray(feat[rows].T),
                "ws": ws_bf,
                "bsT": bsT,
            }
        )

    trace = os.environ.get("K_TRACE", "0") == "1"
    kw = {}
    if trace:
        kw["trace_cores"] = list(range(R))
        kw["stitch_traces"] = os.environ.get("K_STITCH", "0") == "1"
    # rare device-state flake can yield NaNs; retry is host-cheap and a
    # no-op on healthy runs
    for _attempt in range(3):
        LAST_RESULT = run_bass_kernel_spmd(
            nc, in_maps, core_ids=list(range(R)), trace=trace, **kw
        )
        outs = [LAST_RESULT.results[c]["out"] for c in range(R)]  # each [H, P]
        out = np.concatenate([o.T for o in outs], axis=0).astype(np.float32)
        if np.isfinite(out).all():
            break
    return out


if __name__ == "__main__":
    rng = np.random.default_rng(0)
    out = kernel(
        rng.standard_normal((N, 3)).astype(np.float32),
        rng.standard_normal((N, 125)).astype(np.float32),
        rng.random((N, N), dtype=np.float32),
        (rng.standard_normal((L, H, H)) / np.sqrt(H)).astype(np.float32),
        np.zeros((L, H), np.float32),
    )
    print("out", out.shape, out.dtype, float(np.abs(out).mean()))
